# revision 20
# baseline (speedup 1.0000x reference)
"""Trainium2 Bass kernel for nn_AttnAggregator (GNN message passing, 8 cores).

Strategy: data-parallel over queries. Each of the 8 NeuronCores owns 256
queries = 2560 segments. Per core, neighbors are grouped into 20 windows of
128 segments; each window's neighbor list is padded to a fixed number of
128-slot tiles (T, uniform across cores so the SPMD program is identical).

The entity/relation tables are shipped twice: f32 (setup path) and f16. The
main loop gathers neighbor embeddings directly in fp16 (halving gather HBM
traffic), DMA-transposes them for the z GEMM, and keeps every SBUF tensor in
fp16 so DVE ops hit the 2x/4x perf modes and PE matmuls avoid f32r.

The z GEMM runs TRANSPOSED (zT[h, slot] with W1 chunks stationary) so the
score reduction over h becomes per-tile PE dot products against v (1-col
moving operands) instead of DVE tensor work, and exp is one tiny ACT op.

Pipeline per window w (emission is software-pipelined; at iteration `it`):
  S0(it):    dma_gather em16 fp16 (3 parts, SWDGE q0-3) + 2 xbar transposes
             per part -> emT16 [h-chunk, slot] (+ ss/rr gathers every 5 wins)
  A2(it-3):  exp(sc_psum) -> ebuf; wm[t] = (iota==segl)*e  (DVE)
  A1(it-2):  zT = W1.T @ emT + c[q].T (PE fp16, 512-slot PSUM groups) ->
             tanh (ACT, fp16) -> score[t] = tanhT[:,t].T @ v (PE, PSUM)
  PEB(it-3): agg += wm.T @ em16 ; den += wm.T @ ones  (PE fp16 into PSUM)
  C(it-3):   den+inv, recip (DVE); out assembly agg*dinv | ss*mask | rr*mask
             (ACT); DMA out

The c-table c[q] = s_emb[q] @ W2 + r_emb[q] @ W3 + b is computed on-device in
a small fp32 setup phase (gather + PE transpose + matmuls).
"""

import os
import sys

import numpy as np

H = 256
SEQ_LEN = 10
NCORES = 8
WIN = 128  # segments per output window (PSUM partition dim)
SWB = 5    # windows per ss/rr gather batch


def _wrap_idx(idx_lin):
    """Wrap a linear int16 index list for dma_gather: idx i lives at
    [i % 16, i // 16], replicated across the 8 GPSIMD cores (128 rows)."""
    n = len(idx_lin)
    assert n % 16 == 0
    arr = np.asarray(idx_lin, dtype=np.int16).reshape(n // 16, 16).T  # [16, n//16]
    return np.tile(arr, (8, 1)).copy()  # [128, n//16]


def _build_core_data(c, s, r, nbr_ids, seg_ids, QPC, NW):
    """Pure-integer host-side layout work for one core's shard."""
    qlo = c * QPC
    seg_lo = qlo * SEQ_LEN
    seg_hi = (qlo + QPC) * SEQ_LEN
    lo = np.searchsorted(seg_ids, seg_lo, "left")
    hi = np.searchsorted(seg_ids, seg_hi, "left")
    segs = (seg_ids[lo:hi] - seg_lo).astype(np.int64)  # 0 .. QPC*SEQ_LEN-1
    nbrs = nbr_ids[lo:hi].astype(np.int64)

    win_bounds = [np.searchsorted(segs, w * WIN, "left") for w in range(NW + 1)]
    cnts = [win_bounds[w + 1] - win_bounds[w] for w in range(NW)]
    tiles = [max(1, -(-cnt // 128)) for cnt in cnts]
    return segs, nbrs, win_bounds, cnts, tiles


def kernel(s, r, nbr_ids, seg_ids, ent_embeds, rel_embeds, W_attn, b_attn, v_s):
    sys.path.insert(0, "/opt/trn_rl_repo")
    import concourse.bass as bass  # noqa: F401
    import concourse.tile as tile
    from concourse import bacc, mybir
    from concourse.bass_utils import run_bass_kernel_spmd
    from contextlib import ExitStack

    f32 = mybir.dt.float32
    f32r = mybir.dt.float32r
    f16 = mybir.dt.float16
    i16 = mybir.dt.int16
    AF = mybir.ActivationFunctionType
    OP = mybir.AluOpType

    s = np.asarray(s)
    r = np.asarray(r)
    nbr_ids = np.asarray(nbr_ids)
    seg_ids = np.asarray(seg_ids)
    ent_embeds = np.ascontiguousarray(np.asarray(ent_embeds, dtype=np.float32))
    rel_embeds = np.ascontiguousarray(np.asarray(rel_embeds, dtype=np.float32))
    W_attn = np.asarray(W_attn, dtype=np.float32)
    b_attn = np.asarray(b_attn, dtype=np.float32)
    v_s = np.asarray(v_s, dtype=np.float32).reshape(-1)

    ent16_tab = ent_embeds.astype(np.float16)
    rel16_tab = rel_embeds.astype(np.float16)

    B = s.shape[0]
    NUM_SEG = B * SEQ_LEN
    QPC = B // NCORES              # queries per core
    SPC = QPC * SEQ_LEN            # segments per core
    NW = SPC // WIN                # windows per core

    # ---------------- host-side integer layout ----------------
    per_core = [
        _build_core_data(c, s, r, nbr_ids, seg_ids, QPC, NW) for c in range(NCORES)
    ]
    T = max(max(t) for (_, _, _, _, t) in per_core)  # tiles per window (uniform)
    SLOTS = NW * T * 128

    counts_all = np.bincount(np.asarray(seg_ids, dtype=np.int64), minlength=NUM_SEG)

    # Per-WINDOW query base for the c-add one-hot matmul (uniform across
    # cores: computed from w alone). Window w covers local queries
    # [floor(w*128/10), floor(((w+1)*128-1)/10)] — span <= 14 = KQW.
    QW = []  # (qbase, kq) per window
    KQW = WIN // SEQ_LEN + 2  # 14: max queries touched by one window
    for w in range(NW):
        qb = (w * WIN) // SEQ_LEN
        kq = min(KQW, QPC - qb)
        QW.append((qb, kq))

    in_maps = []
    for c in range(NCORES):
        segs, nbrs, wb, cnts, _tiles = per_core[c]
        em_idx = np.zeros(SLOTS, dtype=np.int64)
        segloc = np.full((NW * T, 128), 255.0, dtype=np.float32)  # [tile, part]
        qloc = np.full(SLOTS, -1, dtype=np.int64)
        for w in range(NW):
            cnt = cnts[w]
            base = w * T * 128
            em_idx[base : base + cnt] = nbrs[wb[w] : wb[w + 1]]
            sl = segs[wb[w] : wb[w + 1]] - w * WIN
            tl = np.full(T * 128, 255.0, dtype=np.float32)
            tl[:cnt] = sl.astype(np.float32)
            segloc[w * T : (w + 1) * T, :] = tl.reshape(T, 128)
            qloc[base : base + cnt] = (segs[wb[w] : wb[w + 1]] // SEQ_LEN)

        qoh = np.zeros((KQW, SLOTS), dtype=np.float16)
        for w in range(NW):
            qb = QW[w][0]
            sl = slice(w * T * 128, (w + 1) * T * 128)
            ql = qloc[sl]
            rel_q = np.where(ql >= 0, ql - qb, -1)
            assert rel_q.max() < KQW
            for k in range(KQW):
                qoh[k, sl] = (rel_q == k).astype(np.float16)

        # per-segment arrays
        seg_global0 = c * SPC
        segq = (np.arange(SPC) // SEQ_LEN) + c * QPC  # global query per local seg
        sw_idx = s[segq].astype(np.int64)  # ent row per local seg
        rw_idx = r[segq].astype(np.int64)
        cnts_core = counts_all[seg_global0 : seg_global0 + SPC]
        maskw = (cnts_core > 0).astype(np.float32).reshape(NW, 128).T  # [128, NW]
        invw = 1.0 - maskw

        sq = s[c * QPC : (c + 1) * QPC].astype(np.int64)  # [QPC]
        rq = r[c * QPC : (c + 1) * QPC].astype(np.int64)

        im = {
            "ent": ent_embeds,
            "rel": rel_embeds,
            "ent16": ent16_tab,
            "rel16": rel16_tab,
            "wq1z": W_attn[0:256].reshape(2, 128, 2, 128).transpose(1, 0, 2, 3)
                   .astype(np.float16).copy(),
            "wq2": W_attn[256:512].reshape(2, 128, 256).transpose(1, 0, 2).copy(),
            "wq3": W_attn[512:768].reshape(2, 128, 256).transpose(1, 0, 2).copy(),
            "b_row": b_attn.reshape(1, 256).copy(),
            "vcol": np.ascontiguousarray(
                v_s.astype(np.float16).reshape(2, 128).T),
            "ones2": np.ones((128, 2), dtype=np.float16),
            "ones_row": np.ones((1, 128), dtype=np.float32),
            "ident": np.eye(128, dtype=np.float32),
            "iota32": np.tile(np.arange(128, dtype=np.float32), (128, 1)),
            "em_idx": _wrap_idx(em_idx),
            "sq_idx": _wrap_idx(sq),
            "rq_idx": _wrap_idx(rq),
            "sw_idx": _wrap_idx(sw_idx),
            "rw_idx": _wrap_idx(rw_idx),
            "segl": np.ascontiguousarray(segloc.T),
            "qoh": qoh,
            "maskw": np.ascontiguousarray(maskw),
            "invw": np.ascontiguousarray(invw),
        }
        in_maps.append(im)

    # ---------------- build the SPMD program ----------------
    print("[kernel] host prep done", flush=True)
    nc = bacc.Bacc("TRN2", target_bir_lowering=False, debug=False,
                   num_devices=NCORES, num_swdge_queues=4)

    def din(name, shape, dt):
        return nc.dram_tensor(name, shape, dt, kind="ExternalInput").ap()

    ent_ap = din("ent", [ent_embeds.shape[0], 256], f32)
    rel_ap = din("rel", [rel_embeds.shape[0], 256], f32)
    ent16_ap = din("ent16", [ent_embeds.shape[0], 256], f16)
    rel16_ap = din("rel16", [rel_embeds.shape[0], 256], f16)
    wq1z_ap = din("wq1z", [128, 2, 2, 128], f16)
    wq2_ap = din("wq2", [128, 2, 256], f32)
    wq3_ap = din("wq3", [128, 2, 256], f32)
    brow_ap = din("b_row", [1, 256], f32)
    vcol_ap = din("vcol", [128, 2], f16)
    ones2_ap = din("ones2", [128, 2], f16)
    onesr_ap = din("ones_row", [1, 128], f32)
    ident_ap = din("ident", [128, 128], f32)
    iota32_ap = din("iota32", [128, 128], f32)
    emidx_ap = din("em_idx", [128, SLOTS // 16], i16)
    sqidx_ap = din("sq_idx", [128, QPC // 16], i16)
    rqidx_ap = din("rq_idx", [128, QPC // 16], i16)
    swidx_ap = din("sw_idx", [128, SPC // 16], i16)
    rwidx_ap = din("rw_idx", [128, SPC // 16], i16)
    segl_ap = din("segl", [128, NW * T], f32)
    qoh_ap = din("qoh", [KQW, SLOTS], f16)
    maskw_ap = din("maskw", [128, NW], f32)
    invw_ap = din("invw", [128, NW], f32)
    out_ap = nc.dram_tensor("out", [SPC, 768], f32, kind="ExternalOutput").ap()

    import itertools as _it
    _swq_counter = _it.count()

    def _swq():
        return next(_swq_counter) % 4

    _patch_swdge_lane_assignment()

    import time as _time
    _t0 = _time.time()
    with tile.TileContext(nc) as tc, ExitStack() as ctx:
        cons = ctx.enter_context(tc.tile_pool(name="cons", bufs=1))
        emp = ctx.enter_context(tc.tile_pool(name="emp", bufs=4))
        emq = ctx.enter_context(tc.tile_pool(name="emq", bufs=3))
        hvp = ctx.enter_context(tc.tile_pool(name="hvp", bufs=2))
        wmp = ctx.enter_context(tc.tile_pool(name="wmp", bufs=2))
        work = ctx.enter_context(tc.tile_pool(name="work", bufs=2))
        outp = ctx.enter_context(tc.tile_pool(name="outp", bufs=2))
        ps_z = ctx.enter_context(tc.tile_pool(name="ps_z", bufs=2, space="PSUM"))
        ps_a = ctx.enter_context(tc.tile_pool(name="ps_a", bufs=2, space="PSUM"))
        ps_d = ctx.enter_context(tc.tile_pool(name="ps_d", bufs=1, space="PSUM"))
        ps_s = ctx.enter_context(tc.tile_pool(name="ps_s", bufs=1, space="PSUM"))

        # resident constants
        def cload(tag, shape, dt, ap, cast=False):
            t = cons.tile(shape, dt, tag=tag)
            nc.sync.dma_start(t[:], (ap.bitcast(dt) if cast else ap)[:])
            return t

        wq1z = cload("wq1z", [128, 2, 2, 128], f16, wq1z_ap)
        wq2 = cload("wq2", [128, 2, 256], f32r, wq2_ap, cast=True)
        wq3 = cload("wq3", [128, 2, 256], f32r, wq3_ap, cast=True)
        brow = cload("brow", [1, 256], f32r, brow_ap, cast=True)
        vcol = cload("vcol", [128, 2], f16, vcol_ap)
        onesr = cload("onesr", [1, 128], f32r, onesr_ap, cast=True)
        ident = cload("ident", [128, 128], f32r, ident_ap, cast=True)
        iota32 = cload("iota32", [128, 128], f32, iota32_ap)
        segl = cload("segl", [128, NW * T], f32, segl_ap)
        ones2 = cload("ones2", [128, 2], f16, ones2_ap)
        emidx = cload("emidx", [128, SLOTS // 16], i16, emidx_ap)
        sqidx = cload("sqidx", [128, QPC // 16], i16, sqidx_ap)
        rqidx = cload("rqidx", [128, QPC // 16], i16, rqidx_ap)
        swidx = cload("swidx", [128, SPC // 16], i16, swidx_ap)
        rwidx = cload("rwidx", [128, SPC // 16], i16, rwidx_ap)
        maskw = cload("maskw", [128, NW], f32, maskw_ap)
        invw = cload("invw", [128, NW], f32, invw_ap)

        # ---- setup: c-table c[q] = s_emb[q] @ W2 + r_emb[q] @ W3 + b ----
        # (gathers issued up front; compute emitted after the first two
        # windows' S0 so it overlaps their gather transfers)
        s_emb = cons.tile([128, QPC // 128, 256], f32r)
        r_emb = cons.tile([128, QPC // 128, 256], f32r)

        def setup_gathers():
            nc.gpsimd.dma_gather(s_emb[:], ent_ap.bitcast(f32r)[:], sqidx[:],
                                 num_idxs=QPC, num_idxs_reg=QPC, elem_size=256,
                                 single_packet=False, queue_num=_swq())
            nc.gpsimd.dma_gather(r_emb[:], rel_ap.bitcast(f32r)[:], rqidx[:],
                                 num_idxs=QPC, num_idxs_reg=QPC, elem_size=256,
                                 single_packet=False, queue_num=_swq())

        def setup_ctable():
            sT = cons.tile([128, 2, 256], f32r)   # [h, hc, q]
            rT = cons.tile([128, 2, 256], f32r)
            for gsrc, dstT in ((s_emb, sT), (r_emb, rT)):
                tp = ps_a.tile([128, 2, 256], f32r, tag="agg")
                for qc in range(2):
                    for hc in range(2):
                        nc.tensor.transpose(tp[:, hc, qc * 128:(qc + 1) * 128],
                                            gsrc[:, qc, hc * 128:(hc + 1) * 128],
                                            ident[:])
                nc.scalar.copy(dstT[:], tp[:])

            cw = cons.tile([32, NW, 256], f16)
            for w in range(NW):
                qb, kq = QW[w]
                cp = ps_z.tile([128, 4, 256], f32, tag="z")
                for hc in range(2):
                    nc.tensor.matmul(cp[0:kq, 0, :], sT[:, hc, qb:qb + kq],
                                     wq2[:, hc, :], start=(hc == 0), stop=False)
                for hc in range(2):
                    nc.tensor.matmul(cp[0:kq, 0, :], rT[:, hc, qb:qb + kq],
                                     wq3[:, hc, :], start=False, stop=False)
                nc.tensor.matmul(cp[0:kq, 0, :], onesr[:, 0:kq], brow[:],
                                 start=False, stop=True)
                nc.scalar.copy(cw[0:kq, w, :], cp[0:kq, 0, :])
            return cw

        # ---- software-pipelined main loop over windows ----
        NW_RUN = int(os.environ.get("KERNEL_NWIN", str(NW)))
        NQ = 3
        tparts = [(T * p // NQ, T * (p + 1) // NQ) for p in range(NQ)]
        ngrp = (T + 3) // 4

        em16_t = {}
        emT_t = {}
        qoh_t = {}
        sc_t = {}
        th_t = {}
        wm_t = {}
        agg_t = {}
        den_t = {}
        ssb_t = {}
        rrb_t = {}
        osb_t = {}

        def S0(w):
            """Gather + transpose + one-hot load for window w."""
            em16 = emp.tile([128, T, 256], f16, tag="em16", bufs=5)
            emT = emq.tile([128, T, 2, 128], f16, tag="emT", bufs=4)
            for tlo, thi in tparts:
                nt = thi - tlo
                nc.gpsimd.dma_gather(
                    em16[:, tlo:thi, :], ent16_ap[:],
                    emidx[:, (w * T + tlo) * 8:(w * T + thi) * 8],
                    num_idxs=nt * 128, num_idxs_reg=nt * 128, elem_size=256,
                    single_packet=False, queue_num=_swq())
                nc.sync.dma_start(emT[:, tlo:thi, :, :],
                                  em16[:, tlo:thi, :], transpose=True)
            qoh_w = emq.tile([KQW, T * 128], f16, tag="qoh", bufs=4)
            nc.sync.dma_start(qoh_w[:],
                              qoh_ap[:, w * T * 128:(w + 1) * T * 128])
            em16_t[w] = em16
            emT_t[w] = emT
            qoh_t[w] = qoh_w
            if w % SWB == 0:
                nsw = min(SWB, NW_RUN - w)
                ssb = outp.tile([128, SWB, 256], f16, tag="ssb", bufs=2)
                nc.gpsimd.dma_gather(
                    ssb[:, 0:nsw, :], ent16_ap[:],
                    swidx[:, w * 8:(w + nsw) * 8],
                    num_idxs=nsw * 128, num_idxs_reg=nsw * 128, elem_size=256,
                    single_packet=False, queue_num=_swq())
                rrb = outp.tile([128, SWB, 256], f16, tag="rrb", bufs=2)
                nc.gpsimd.dma_gather(
                    rrb[:, 0:nsw, :], rel16_ap[:],
                    rwidx[:, w * 8:(w + nsw) * 8],
                    num_idxs=nsw * 128, num_idxs_reg=nsw * 128, elem_size=256,
                    single_packet=False, queue_num=_swq())
                ssb_t[w // SWB] = ssb
                rrb_t[w // SWB] = rrb

        def A1(w):
            """zT GEMM + tanh + PE score dot-products for window w."""
            qb, kq = QW[w]
            emT = emT_t.pop(w)
            qoh_w = qoh_t.pop(w)
            tanhT = hvp.tile([128, 2, T * 128], f16, tag="tanhT", bufs=2)
            sc_ps = ps_s.tile([128, T], f32, tag="sc")

            def score_mm(g):
                t0 = g * 4
                for t in range(t0, min(t0 + 4, T)):
                    for hc in range(2):
                        nc.tensor.matmul(sc_ps[:, t:t + 1],
                                         tanhT[:, hc, t * 128:(t + 1) * 128],
                                         vcol[:, hc:hc + 1],
                                         start=(hc == 0), stop=(hc == 1))

            for g in range(ngrp):
                t0 = g * 4
                nt = min(4, T - t0)
                sl = slice(t0 * 128, (t0 + nt) * 128)
                zp = ps_z.tile([128, 2, 512], f32, tag="z")
                for hc in range(2):
                    zps = zp[:, hc, 0:nt * 128]
                    for kc in range(2):
                        nc.tensor.matmul(zps, wq1z[:, kc, hc, :],
                                         emT[:, t0:t0 + nt, kc, :],
                                         start=(kc == 0), stop=False)
                    nc.tensor.matmul(zps,
                                     c_win[0:kq, w, hc * 128:(hc + 1) * 128],
                                     qoh_w[0:kq, sl],
                                     start=False, stop=True)
                nc.scalar.activation(tanhT[:, :, sl], zp[:, :, 0:nt * 128],
                                     AF.Tanh)
            th_t[w] = (tanhT, sc_ps, score_mm)

        def A1s(w):
            """Score dot-products for window w (emitted after PEB(w-1) so the
            agg matmuls hide the tanh latency)."""
            tanhT, sc_ps, score_mm = th_t.pop(w)
            for g in range(ngrp):
                score_mm(g)
            sc_t[w] = sc_ps

        def A2(w):
            """exp + per-tile weight-mask generation for window w."""
            sc_ps = sc_t.pop(w)
            ebuf = wmp.tile([128, T], f32, tag="ebuf", bufs=2)
            nc.scalar.activation(ebuf[:], sc_ps[:], AF.Exp)
            wm = wmp.tile([128, T, 128], f16, tag="wm", bufs=2)
            for t in range(T):
                e_b, _ = bass.broadcast_tensor_aps(ebuf[:, t:t + 1], iota32[:])
                nc.vector.scalar_tensor_tensor(
                    wm[:, t, :], iota32[:],
                    segl[:, w * T + t:w * T + t + 1], e_b,
                    op0=OP.is_equal, op1=OP.mult)
            wm_t[w] = wm

        def PEB(w):
            """Scatter matmuls for window w."""
            wm = wm_t.pop(w)
            em16 = em16_t.pop(w)
            agg_ps = ps_a.tile([128, 256], f32, tag="agg")
            den_ps = ps_d.tile([128, 2], f32, tag="den")
            for t in range(T):
                nc.tensor.matmul(agg_ps[:], wm[:, t, :], em16[:, t, :],
                                 start=(t == 0), stop=(t == T - 1))
                nc.tensor.matmul(den_ps[:], wm[:, t, :], ones2[:],
                                 start=(t == 0), stop=(t == T - 1))
            agg_t[w] = agg_ps
            den_t[w] = den_ps

        def C(w):
            """Normalize + assemble + write out window w."""
            agg_ps = agg_t.pop(w)
            den_ps = den_t.pop(w)
            dtmp = work.tile([128, 1], f32, tag="dtmp", bufs=2)
            nc.vector.tensor_add(dtmp[:], den_ps[:, 0:1], invw[:, w:w + 1])
            dinv = work.tile([128, 1], f32, tag="dinv", bufs=2)
            nc.vector.reciprocal(dinv[:], dtmp[:])
            out_sb = outp.tile([128, 768], f32, tag="out", bufs=4)
            nc.scalar.activation(out_sb[:, 0:256], agg_ps[:, 0:256], AF.Copy,
                                 scale=dinv[:])
            b, k = w // SWB, w % SWB
            nc.scalar.activation(out_sb[:, 256:512], ssb_t[b][:, k, :],
                                 AF.Copy, scale=maskw[:, w:w + 1])
            nc.scalar.activation(out_sb[:, 512:768], rrb_t[b][:, k, :],
                                 AF.Copy, scale=maskw[:, w:w + 1])
            osb_t[w] = out_sb

        def CDMA(w):
            """Write out window w (emitted before next transposes on Sync)."""
            out_sb = osb_t.pop(w)
            nc.sync.dma_start(out_ap[w * 128:(w + 1) * 128, :], out_sb[:])

        c_win = None
        for it in range(NW_RUN + 5):
            if 0 <= it - 4 < NW_RUN:
                CDMA(it - 4)
            if it < NW_RUN:
                S0(it)
            if it == 0:
                setup_gathers()
            if it == min(1, NW_RUN - 1):
                c_win = setup_ctable()
            if 0 <= it - 3 < NW_RUN:
                A2(it - 3)
            if 0 <= it - 2 < NW_RUN:
                A1(it - 2)
            if 0 <= it - 3 < NW_RUN:
                PEB(it - 3)
            if 0 <= it - 2 < NW_RUN:
                A1s(it - 2)
            if 0 <= it - 3 < NW_RUN:
                C(it - 3)

    print(f"[kernel] program built+scheduled in {_time.time()-_t0:.1f}s",
          flush=True)
    nc.compile()
    print("[kernel] bacc.compile done; launching", flush=True)

    if os.environ.get("KERNEL_SIM"):
        from concourse.bass_interp import CoreSim
        sim = CoreSim(nc, trace=False)
        for k, v in in_maps[0].items():
            sim.tensor(k)[:] = v
        sim.simulate(check_with_hw=False)
        print("[kernel] CoreSim passed", flush=True)
        import types
        res = types.SimpleNamespace(
            results=[{"out": np.array(sim.tensor("out"))} for _ in range(NCORES)],
            exec_time_ns=None)
        out = np.concatenate([res.results[c]["out"] for c in range(NCORES)], axis=0)
        return out.reshape(B, SEQ_LEN, 3 * H)

    trace = bool(int(os.environ.get("KERNEL_TRACE", "0")))
    if trace:
        _install_prof_hook()
    res = run_bass_kernel_spmd(nc, in_maps, list(range(NCORES)), trace=trace)
    if trace and res.exec_time_ns is not None:
        print(f"HW exec time: {res.exec_time_ns} ns")

    out = np.concatenate([res.results[c]["out"] for c in range(NCORES)], axis=0)
    return out.reshape(B, SEQ_LEN, 3 * H)


def _patch_swdge_lane_assignment():
    """Make Tile's DMASW completion-sem lane choice queue-aware so SWDGE
    multi-queue DMAs don't share a semaphore lane across queues (each sem is
    locked to the queue that first increments it). Lanes 2q and 2q+1 serve
    queue q."""
    import concourse.tile_sem_assignment as tsa
    import concourse.mybir as mybir

    cls = tsa.TileClockTick
    if getattr(cls, "_swq_patched", False):
        return
    orig = cls._assign_tick

    def _assign_tick(self, inst):
        if (
            isinstance(inst, tsa.DMAInst)
            and inst.engine == mybir.EngineType.Pool
        ):
            q = getattr(inst, "queue_num", 0) or 0
            if not hasattr(self, "_swq_rot"):
                self._swq_rot = {}
            rot = self._swq_rot.get(q, 0)
            self._swq_rot[q] = rot ^ 1
            lane = (2 * q + rot) % self.swdge_sem_count
            save = self.next_sw_dma_idx
            self.next_sw_dma_idx = lane
            try:
                return orig(self, inst)
            finally:
                self.next_sw_dma_idx = save
        return orig(self, inst)

    cls._assign_tick = _assign_tick
    cls._swq_patched = True


def _install_prof_hook():
    """Shim antenv.axon_hooks so trace=True can NTFF-profile under axon."""
    import contextlib
    import ctypes
    import types

    import antenv

    if "antenv.axon_hooks" in sys.modules:
        return
    so = "/opt/axon/libaxon_pjrt.so"
    lib = ctypes.CDLL(so)
    if not hasattr(lib, "axon_start_nrt_profile"):
        return
    lib.axon_start_nrt_profile.argtypes = [ctypes.POINTER(ctypes.c_int64),
                                           ctypes.c_size_t]
    lib.axon_start_nrt_profile.restype = ctypes.c_int64
    lib.axon_stop_nrt_profile.argtypes = [ctypes.c_char_p]
    lib.axon_stop_nrt_profile.restype = ctypes.c_int64

    @contextlib.contextmanager
    def _hook(output_dir, device_ids):
        import jax

        jax.devices()
        if device_ids:
            ids = (ctypes.c_int64 * len(device_ids))(*device_ids)
            rc = lib.axon_start_nrt_profile(ids, len(device_ids))
        else:
            rc = lib.axon_start_nrt_profile(None, 0)
        if rc != 0:
            raise RuntimeError(f"axon_start_nrt_profile rc={rc}")
        try:
            yield
        finally:
            n = lib.axon_stop_nrt_profile(str(output_dir).encode())
            print(f"profile: {n} file(s) written to {output_dir}",
                  file=sys.stderr)

    mod = types.ModuleType("antenv.axon_hooks")
    mod.get_axon_ntff_profile_hook = lambda: _hook
    mod.set_axon_ntff_profile_hook = lambda h: None
    sys.modules["antenv.axon_hooks"] = mod
    antenv.axon_hooks = mod


# revision 21
# speedup vs baseline: 1.0856x; 1.0856x over previous
"""Trainium2 Bass kernel for nn_AttnAggregator (GNN message passing, 8 cores).

Strategy: data-parallel over queries. Each of the 8 NeuronCores owns 256
queries = 2560 segments. Per core, neighbors are grouped into 20 windows of
128 segments; each window's neighbor list is padded to a fixed number of
128-slot tiles (T, uniform across cores so the SPMD program is identical).

The entity/relation tables are shipped twice: f32 (setup path) and f16. The
main loop gathers neighbor embeddings directly in fp16 (halving gather HBM
traffic), DMA-transposes them for the z GEMM, and keeps every SBUF tensor in
fp16 so DVE ops hit the 2x/4x perf modes and PE matmuls avoid f32r.

The z GEMM runs TRANSPOSED (zT[h, slot] with W1 chunks stationary) so the
score reduction over h becomes per-tile PE dot products against v (1-col
moving operands) instead of DVE tensor work, and exp is one tiny ACT op.

Pipeline per window w (emission is software-pipelined; at iteration `it`):
  S0(it):    dma_gather em16 fp16 (3 parts, SWDGE q0-3) + 2 xbar transposes
             per part -> emT16 [h-chunk, slot] (+ ss/rr gathers every 5 wins)
  A2(it-3):  exp(sc_psum) -> ebuf; wm[t] = (iota==segl)*e  (DVE)
  A1(it-2):  zT = W1.T @ emT + c[q].T (PE fp16, 512-slot PSUM groups) ->
             tanh (ACT, fp16) -> score[t] = tanhT[:,t].T @ v (PE, PSUM)
  PEB(it-3): agg += wm.T @ em16 ; den += wm.T @ ones  (PE fp16 into PSUM)
  C(it-3):   den+inv, recip (DVE); out assembly agg*dinv | ss*mask | rr*mask
             (ACT); DMA out

The c-table c[q] = s_emb[q] @ W2 + r_emb[q] @ W3 + b is computed on-device in
a small fp32 setup phase (gather + PE transpose + matmuls).
"""

import os
import sys

import numpy as np

H = 256
SEQ_LEN = 10
NCORES = 8
WIN = 128  # segments per output window (PSUM partition dim)
SWB = 5    # windows per ss/rr gather batch


def _wrap_idx(idx_lin):
    """Wrap a linear int16 index list for dma_gather: idx i lives at
    [i % 16, i // 16], replicated across the 8 GPSIMD cores (128 rows)."""
    n = len(idx_lin)
    assert n % 16 == 0
    arr = np.asarray(idx_lin, dtype=np.int16).reshape(n // 16, 16).T  # [16, n//16]
    return np.tile(arr, (8, 1)).copy()  # [128, n//16]


def _build_core_data(c, s, r, nbr_ids, seg_ids, QPC, NW):
    """Pure-integer host-side layout work for one core's shard."""
    qlo = c * QPC
    seg_lo = qlo * SEQ_LEN
    seg_hi = (qlo + QPC) * SEQ_LEN
    lo = np.searchsorted(seg_ids, seg_lo, "left")
    hi = np.searchsorted(seg_ids, seg_hi, "left")
    segs = (seg_ids[lo:hi] - seg_lo).astype(np.int64)  # 0 .. QPC*SEQ_LEN-1
    nbrs = nbr_ids[lo:hi].astype(np.int64)

    win_bounds = [np.searchsorted(segs, w * WIN, "left") for w in range(NW + 1)]
    cnts = [win_bounds[w + 1] - win_bounds[w] for w in range(NW)]
    tiles = [max(1, -(-cnt // 128)) for cnt in cnts]
    return segs, nbrs, win_bounds, cnts, tiles


def kernel(s, r, nbr_ids, seg_ids, ent_embeds, rel_embeds, W_attn, b_attn, v_s):
    sys.path.insert(0, "/opt/trn_rl_repo")
    import concourse.bass as bass  # noqa: F401
    import concourse.tile as tile
    from concourse import bacc, mybir
    from concourse.bass_utils import run_bass_kernel_spmd
    from contextlib import ExitStack

    f32 = mybir.dt.float32
    f32r = mybir.dt.float32r
    f16 = mybir.dt.float16
    i16 = mybir.dt.int16
    AF = mybir.ActivationFunctionType
    OP = mybir.AluOpType

    s = np.asarray(s)
    r = np.asarray(r)
    nbr_ids = np.asarray(nbr_ids)
    seg_ids = np.asarray(seg_ids)
    ent_embeds = np.ascontiguousarray(np.asarray(ent_embeds, dtype=np.float32))
    rel_embeds = np.ascontiguousarray(np.asarray(rel_embeds, dtype=np.float32))
    W_attn = np.asarray(W_attn, dtype=np.float32)
    b_attn = np.asarray(b_attn, dtype=np.float32)
    v_s = np.asarray(v_s, dtype=np.float32).reshape(-1)

    ent16_tab = ent_embeds.astype(np.float16)
    rel16_tab = rel_embeds.astype(np.float16)

    B = s.shape[0]
    NUM_SEG = B * SEQ_LEN
    QPC = B // NCORES              # queries per core
    SPC = QPC * SEQ_LEN            # segments per core
    NW = SPC // WIN                # windows per core

    # ---------------- host-side integer layout ----------------
    per_core = [
        _build_core_data(c, s, r, nbr_ids, seg_ids, QPC, NW) for c in range(NCORES)
    ]
    T = max(max(t) for (_, _, _, _, t) in per_core)  # tiles per window (uniform)
    SLOTS = NW * T * 128

    counts_all = np.bincount(np.asarray(seg_ids, dtype=np.int64), minlength=NUM_SEG)

    # Per-WINDOW query base for the c-add one-hot matmul (uniform across
    # cores: computed from w alone). Window w covers local queries
    # [floor(w*128/10), floor(((w+1)*128-1)/10)] — span <= 14 = KQW.
    QW = []  # (qbase, kq) per window
    KQW = WIN // SEQ_LEN + 2  # 14: max queries touched by one window
    for w in range(NW):
        qb = (w * WIN) // SEQ_LEN
        kq = min(KQW, QPC - qb)
        QW.append((qb, kq))

    in_maps = []
    for c in range(NCORES):
        segs, nbrs, wb, cnts, _tiles = per_core[c]
        em_idx = np.zeros(SLOTS, dtype=np.int64)
        segloc = np.full((NW * T, 128), 255.0, dtype=np.float32)  # [tile, part]
        qloc = np.full(SLOTS, -1, dtype=np.int64)
        for w in range(NW):
            cnt = cnts[w]
            base = w * T * 128
            em_idx[base : base + cnt] = nbrs[wb[w] : wb[w + 1]]
            sl = segs[wb[w] : wb[w + 1]] - w * WIN
            tl = np.full(T * 128, 255.0, dtype=np.float32)
            tl[:cnt] = sl.astype(np.float32)
            segloc[w * T : (w + 1) * T, :] = tl.reshape(T, 128)
            qloc[base : base + cnt] = (segs[wb[w] : wb[w + 1]] // SEQ_LEN)

        qoh = np.zeros((KQW, SLOTS), dtype=np.float16)
        for w in range(NW):
            qb = QW[w][0]
            sl = slice(w * T * 128, (w + 1) * T * 128)
            ql = qloc[sl]
            rel_q = np.where(ql >= 0, ql - qb, -1)
            assert rel_q.max() < KQW
            for k in range(KQW):
                qoh[k, sl] = (rel_q == k).astype(np.float16)

        # per-segment arrays
        seg_global0 = c * SPC
        segq = (np.arange(SPC) // SEQ_LEN) + c * QPC  # global query per local seg
        sw_idx = s[segq].astype(np.int64)  # ent row per local seg
        rw_idx = r[segq].astype(np.int64)
        cnts_core = counts_all[seg_global0 : seg_global0 + SPC]
        maskw = (cnts_core > 0).astype(np.float32).reshape(NW, 128).T  # [128, NW]
        invw = 1.0 - maskw

        sq = s[c * QPC : (c + 1) * QPC].astype(np.int64)  # [QPC]
        rq = r[c * QPC : (c + 1) * QPC].astype(np.int64)

        im = {
            "ent": ent_embeds,
            "rel": rel_embeds,
            "ent16": ent16_tab,
            "rel16": rel16_tab,
            "wq1z": W_attn[0:256].reshape(2, 128, 2, 128).transpose(1, 0, 2, 3)
                   .astype(np.float16).copy(),
            "wq2": W_attn[256:512].reshape(2, 128, 256).transpose(1, 0, 2).copy(),
            "wq3": W_attn[512:768].reshape(2, 128, 256).transpose(1, 0, 2).copy(),
            "b_row": b_attn.reshape(1, 256).copy(),
            "vcol": np.ascontiguousarray(
                v_s.astype(np.float16).reshape(2, 128).T),
            "ones2": np.ones((128, 2), dtype=np.float16),
            "ones_row": np.ones((1, 128), dtype=np.float32),
            "ident": np.eye(128, dtype=np.float32),
            "iota32": np.tile(np.arange(128, dtype=np.float32), (128, 1)),
            "em_idx": _wrap_idx(em_idx),
            "sq_idx": _wrap_idx(sq),
            "rq_idx": _wrap_idx(rq),
            "sw_idx": _wrap_idx(sw_idx),
            "rw_idx": _wrap_idx(rw_idx),
            "segl": np.ascontiguousarray(segloc.T),
            "qoh": qoh,
            "maskw": np.ascontiguousarray(maskw),
            "invw": np.ascontiguousarray(invw),
        }
        in_maps.append(im)

    # ---------------- build the SPMD program ----------------
    print("[kernel] host prep done", flush=True)
    nc = bacc.Bacc("TRN2", target_bir_lowering=False, debug=False,
                   num_devices=NCORES, num_swdge_queues=4)

    def din(name, shape, dt):
        return nc.dram_tensor(name, shape, dt, kind="ExternalInput").ap()

    ent_ap = din("ent", [ent_embeds.shape[0], 256], f32)
    rel_ap = din("rel", [rel_embeds.shape[0], 256], f32)
    ent16_ap = din("ent16", [ent_embeds.shape[0], 256], f16)
    rel16_ap = din("rel16", [rel_embeds.shape[0], 256], f16)
    wq1z_ap = din("wq1z", [128, 2, 2, 128], f16)
    wq2_ap = din("wq2", [128, 2, 256], f32)
    wq3_ap = din("wq3", [128, 2, 256], f32)
    brow_ap = din("b_row", [1, 256], f32)
    vcol_ap = din("vcol", [128, 2], f16)
    ones2_ap = din("ones2", [128, 2], f16)
    onesr_ap = din("ones_row", [1, 128], f32)
    ident_ap = din("ident", [128, 128], f32)
    iota32_ap = din("iota32", [128, 128], f32)
    emidx_ap = din("em_idx", [128, SLOTS // 16], i16)
    sqidx_ap = din("sq_idx", [128, QPC // 16], i16)
    rqidx_ap = din("rq_idx", [128, QPC // 16], i16)
    swidx_ap = din("sw_idx", [128, SPC // 16], i16)
    rwidx_ap = din("rw_idx", [128, SPC // 16], i16)
    segl_ap = din("segl", [128, NW * T], f32)
    qoh_ap = din("qoh", [KQW, SLOTS], f16)
    maskw_ap = din("maskw", [128, NW], f32)
    invw_ap = din("invw", [128, NW], f32)
    out_ap = nc.dram_tensor("out", [SPC, 768], f32, kind="ExternalOutput").ap()

    import itertools as _it
    _swq_counter = _it.count()

    def _swq():
        return next(_swq_counter) % 4

    _patch_swdge_lane_assignment()

    import time as _time
    _t0 = _time.time()
    with tile.TileContext(nc) as tc, ExitStack() as ctx:
        cons = ctx.enter_context(tc.tile_pool(name="cons", bufs=1))
        emp = ctx.enter_context(tc.tile_pool(name="emp", bufs=4))
        emq = ctx.enter_context(tc.tile_pool(name="emq", bufs=3))
        hvp = ctx.enter_context(tc.tile_pool(name="hvp", bufs=2))
        wmp = ctx.enter_context(tc.tile_pool(name="wmp", bufs=2))
        work = ctx.enter_context(tc.tile_pool(name="work", bufs=2))
        outp = ctx.enter_context(tc.tile_pool(name="outp", bufs=2))
        ps_z = ctx.enter_context(tc.tile_pool(name="ps_z", bufs=2, space="PSUM"))
        ps_a = ctx.enter_context(tc.tile_pool(name="ps_a", bufs=2, space="PSUM"))
        ps_d = ctx.enter_context(tc.tile_pool(name="ps_d", bufs=1, space="PSUM"))
        ps_s = ctx.enter_context(tc.tile_pool(name="ps_s", bufs=1, space="PSUM"))

        # resident constants
        def cload(tag, shape, dt, ap, cast=False):
            t = cons.tile(shape, dt, tag=tag)
            nc.sync.dma_start(t[:], (ap.bitcast(dt) if cast else ap)[:])
            return t

        wq1z = cload("wq1z", [128, 2, 2, 128], f16, wq1z_ap)
        wq2 = cload("wq2", [128, 2, 256], f32r, wq2_ap, cast=True)
        wq3 = cload("wq3", [128, 2, 256], f32r, wq3_ap, cast=True)
        brow = cload("brow", [1, 256], f32r, brow_ap, cast=True)
        vcol = cload("vcol", [128, 2], f16, vcol_ap)
        onesr = cload("onesr", [1, 128], f32r, onesr_ap, cast=True)
        ident = cload("ident", [128, 128], f32r, ident_ap, cast=True)
        iota32 = cload("iota32", [128, 128], f32, iota32_ap)
        segl = cload("segl", [128, NW * T], f32, segl_ap)
        ones2 = cload("ones2", [128, 2], f16, ones2_ap)
        emidx = cload("emidx", [128, SLOTS // 16], i16, emidx_ap)
        sqidx = cload("sqidx", [128, QPC // 16], i16, sqidx_ap)
        rqidx = cload("rqidx", [128, QPC // 16], i16, rqidx_ap)
        swidx = cload("swidx", [128, SPC // 16], i16, swidx_ap)
        rwidx = cload("rwidx", [128, SPC // 16], i16, rwidx_ap)
        maskw = cload("maskw", [128, NW], f32, maskw_ap)
        invw = cload("invw", [128, NW], f32, invw_ap)

        # ---- setup: c-table c[q] = s_emb[q] @ W2 + r_emb[q] @ W3 + b ----
        # (gathers issued up front; compute emitted after the first two
        # windows' S0 so it overlaps their gather transfers)
        s_emb = cons.tile([128, QPC // 128, 256], f32r)
        nc.gpsimd.dma_gather(s_emb[:], ent_ap.bitcast(f32r)[:], sqidx[:],
                             num_idxs=QPC, num_idxs_reg=QPC, elem_size=256,
                             single_packet=False, queue_num=_swq())
        r_emb = cons.tile([128, QPC // 128, 256], f32r)
        nc.gpsimd.dma_gather(r_emb[:], rel_ap.bitcast(f32r)[:], rqidx[:],
                             num_idxs=QPC, num_idxs_reg=QPC, elem_size=256,
                             single_packet=False, queue_num=_swq())

        def setup_ctable():
            sT = cons.tile([128, 2, 256], f32r)   # [h, hc, q]
            rT = cons.tile([128, 2, 256], f32r)
            for gsrc, dstT in ((s_emb, sT), (r_emb, rT)):
                tp = ps_a.tile([128, 2, 256], f32r, tag="agg")
                for qc in range(2):
                    for hc in range(2):
                        nc.tensor.transpose(tp[:, hc, qc * 128:(qc + 1) * 128],
                                            gsrc[:, qc, hc * 128:(hc + 1) * 128],
                                            ident[:])
                nc.scalar.copy(dstT[:], tp[:])

            cw = cons.tile([32, NW, 256], f16)
            for w in range(NW):
                qb, kq = QW[w]
                cp = ps_z.tile([128, 4, 256], f32, tag="z")
                for hc in range(2):
                    nc.tensor.matmul(cp[0:kq, 0, :], sT[:, hc, qb:qb + kq],
                                     wq2[:, hc, :], start=(hc == 0), stop=False)
                for hc in range(2):
                    nc.tensor.matmul(cp[0:kq, 0, :], rT[:, hc, qb:qb + kq],
                                     wq3[:, hc, :], start=False, stop=False)
                nc.tensor.matmul(cp[0:kq, 0, :], onesr[:, 0:kq], brow[:],
                                 start=False, stop=True)
                nc.scalar.copy(cw[0:kq, w, :], cp[0:kq, 0, :])
            return cw

        # ---- software-pipelined main loop over windows ----
        NW_RUN = int(os.environ.get("KERNEL_NWIN", str(NW)))
        NQ = 3
        tparts = [(T * p // NQ, T * (p + 1) // NQ) for p in range(NQ)]
        ngrp = (T + 3) // 4

        em16_t = {}
        emT_t = {}
        qoh_t = {}
        sc_t = {}
        th_t = {}
        wm_t = {}
        agg_t = {}
        den_t = {}
        ssb_t = {}
        rrb_t = {}
        osb_t = {}

        def S0(w):
            """Gather + transpose + one-hot load for window w."""
            em16 = emp.tile([128, T, 256], f16, tag="em16", bufs=5)
            emT = emq.tile([128, T, 2, 128], f16, tag="emT", bufs=4)
            for tlo, thi in tparts:
                nt = thi - tlo
                nc.gpsimd.dma_gather(
                    em16[:, tlo:thi, :], ent16_ap[:],
                    emidx[:, (w * T + tlo) * 8:(w * T + thi) * 8],
                    num_idxs=nt * 128, num_idxs_reg=nt * 128, elem_size=256,
                    single_packet=False, queue_num=_swq())
                nc.sync.dma_start(emT[:, tlo:thi, :, :],
                                  em16[:, tlo:thi, :], transpose=True)
            qoh_w = emq.tile([KQW, T * 128], f16, tag="qoh", bufs=4)
            nc.sync.dma_start(qoh_w[:],
                              qoh_ap[:, w * T * 128:(w + 1) * T * 128])
            em16_t[w] = em16
            emT_t[w] = emT
            qoh_t[w] = qoh_w
            if w % SWB == 0:
                nsw = min(SWB, NW_RUN - w)
                ssb = outp.tile([128, SWB, 256], f16, tag="ssb", bufs=2)
                nc.gpsimd.dma_gather(
                    ssb[:, 0:nsw, :], ent16_ap[:],
                    swidx[:, w * 8:(w + nsw) * 8],
                    num_idxs=nsw * 128, num_idxs_reg=nsw * 128, elem_size=256,
                    single_packet=False, queue_num=_swq())
                rrb = outp.tile([128, SWB, 256], f16, tag="rrb", bufs=2)
                nc.gpsimd.dma_gather(
                    rrb[:, 0:nsw, :], rel16_ap[:],
                    rwidx[:, w * 8:(w + nsw) * 8],
                    num_idxs=nsw * 128, num_idxs_reg=nsw * 128, elem_size=256,
                    single_packet=False, queue_num=_swq())
                ssb_t[w // SWB] = ssb
                rrb_t[w // SWB] = rrb

        def A1(w):
            """zT GEMM + tanh + PE score dot-products for window w."""
            qb, kq = QW[w]
            emT = emT_t.pop(w)
            qoh_w = qoh_t.pop(w)
            tanhT = hvp.tile([128, 2, T * 128], f16, tag="tanhT", bufs=2)
            sc_ps = ps_s.tile([128, T], f32, tag="sc")

            def score_mm(g):
                t0 = g * 4
                for t in range(t0, min(t0 + 4, T)):
                    for hc in range(2):
                        nc.tensor.matmul(sc_ps[:, t:t + 1],
                                         tanhT[:, hc, t * 128:(t + 1) * 128],
                                         vcol[:, hc:hc + 1],
                                         start=(hc == 0), stop=(hc == 1))

            for g in range(ngrp):
                t0 = g * 4
                nt = min(4, T - t0)
                sl = slice(t0 * 128, (t0 + nt) * 128)
                zp = ps_z.tile([128, 2, 512], f32, tag="z")
                for hc in range(2):
                    zps = zp[:, hc, 0:nt * 128]
                    for kc in range(2):
                        nc.tensor.matmul(zps, wq1z[:, kc, hc, :],
                                         emT[:, t0:t0 + nt, kc, :],
                                         start=(kc == 0), stop=False)
                    nc.tensor.matmul(zps,
                                     c_win[0:kq, w, hc * 128:(hc + 1) * 128],
                                     qoh_w[0:kq, sl],
                                     start=False, stop=True)
                nc.scalar.activation(tanhT[:, :, sl], zp[:, :, 0:nt * 128],
                                     AF.Tanh)
            th_t[w] = (tanhT, sc_ps, score_mm)

        def A1s(w):
            """Score dot-products for window w (emitted after PEB(w-1) so the
            agg matmuls hide the tanh latency)."""
            tanhT, sc_ps, score_mm = th_t.pop(w)
            for g in range(ngrp):
                score_mm(g)
            sc_t[w] = sc_ps

        def A2(w):
            """exp + per-tile weight-mask generation for window w."""
            sc_ps = sc_t.pop(w)
            ebuf = wmp.tile([128, T], f32, tag="ebuf", bufs=2)
            nc.scalar.activation(ebuf[:], sc_ps[:], AF.Exp)
            wm = wmp.tile([128, T, 128], f16, tag="wm", bufs=2)
            for t in range(T):
                e_b, _ = bass.broadcast_tensor_aps(ebuf[:, t:t + 1], iota32[:])
                nc.vector.scalar_tensor_tensor(
                    wm[:, t, :], iota32[:],
                    segl[:, w * T + t:w * T + t + 1], e_b,
                    op0=OP.is_equal, op1=OP.mult)
            wm_t[w] = wm

        def PEB(w):
            """Scatter matmuls for window w."""
            wm = wm_t.pop(w)
            em16 = em16_t.pop(w)
            agg_ps = ps_a.tile([128, 256], f32, tag="agg")
            den_ps = ps_d.tile([128, 2], f32, tag="den")
            for t in range(T):
                nc.tensor.matmul(agg_ps[:], wm[:, t, :], em16[:, t, :],
                                 start=(t == 0), stop=(t == T - 1))
                nc.tensor.matmul(den_ps[:], wm[:, t, :], ones2[:],
                                 start=(t == 0), stop=(t == T - 1))
            agg_t[w] = agg_ps
            den_t[w] = den_ps

        def C(w):
            """Normalize + assemble + write out window w."""
            agg_ps = agg_t.pop(w)
            den_ps = den_t.pop(w)
            dtmp = work.tile([128, 1], f32, tag="dtmp", bufs=2)
            nc.vector.tensor_add(dtmp[:], den_ps[:, 0:1], invw[:, w:w + 1])
            dinv = work.tile([128, 1], f32, tag="dinv", bufs=2)
            nc.vector.reciprocal(dinv[:], dtmp[:])
            out_sb = outp.tile([128, 768], f32, tag="out", bufs=4)
            nc.scalar.activation(out_sb[:, 0:256], agg_ps[:, 0:256], AF.Copy,
                                 scale=dinv[:])
            b, k = w // SWB, w % SWB
            nc.scalar.activation(out_sb[:, 256:512], ssb_t[b][:, k, :],
                                 AF.Copy, scale=maskw[:, w:w + 1])
            nc.scalar.activation(out_sb[:, 512:768], rrb_t[b][:, k, :],
                                 AF.Copy, scale=maskw[:, w:w + 1])
            osb_t[w] = out_sb

        def CDMA(w):
            """Write out window w (emitted before next transposes on Sync)."""
            out_sb = osb_t.pop(w)
            nc.sync.dma_start(out_ap[w * 128:(w + 1) * 128, :], out_sb[:])

        c_win = None
        for it in range(NW_RUN + 5):
            if 0 <= it - 4 < NW_RUN:
                CDMA(it - 4)
            if it < NW_RUN:
                S0(it)
            if it == min(1, NW_RUN - 1):
                c_win = setup_ctable()
            if 0 <= it - 3 < NW_RUN:
                A2(it - 3)
            if 0 <= it - 2 < NW_RUN:
                A1(it - 2)
            if 0 <= it - 3 < NW_RUN:
                PEB(it - 3)
            if 0 <= it - 2 < NW_RUN:
                A1s(it - 2)
            if 0 <= it - 3 < NW_RUN:
                C(it - 3)

    print(f"[kernel] program built+scheduled in {_time.time()-_t0:.1f}s",
          flush=True)
    nc.compile()
    print("[kernel] bacc.compile done; launching", flush=True)

    if os.environ.get("KERNEL_SIM"):
        from concourse.bass_interp import CoreSim
        sim = CoreSim(nc, trace=False)
        for k, v in in_maps[0].items():
            sim.tensor(k)[:] = v
        sim.simulate(check_with_hw=False)
        print("[kernel] CoreSim passed", flush=True)
        import types
        res = types.SimpleNamespace(
            results=[{"out": np.array(sim.tensor("out"))} for _ in range(NCORES)],
            exec_time_ns=None)
        out = np.concatenate([res.results[c]["out"] for c in range(NCORES)], axis=0)
        return out.reshape(B, SEQ_LEN, 3 * H)

    trace = bool(int(os.environ.get("KERNEL_TRACE", "0")))
    if trace:
        _install_prof_hook()
    res = run_bass_kernel_spmd(nc, in_maps, list(range(NCORES)), trace=trace)
    if trace and res.exec_time_ns is not None:
        print(f"HW exec time: {res.exec_time_ns} ns")

    out = np.concatenate([res.results[c]["out"] for c in range(NCORES)], axis=0)
    return out.reshape(B, SEQ_LEN, 3 * H)


def _patch_swdge_lane_assignment():
    """Make Tile's DMASW completion-sem lane choice queue-aware so SWDGE
    multi-queue DMAs don't share a semaphore lane across queues (each sem is
    locked to the queue that first increments it). Lanes 2q and 2q+1 serve
    queue q."""
    import concourse.tile_sem_assignment as tsa
    import concourse.mybir as mybir

    cls = tsa.TileClockTick
    if getattr(cls, "_swq_patched", False):
        return
    orig = cls._assign_tick

    def _assign_tick(self, inst):
        if (
            isinstance(inst, tsa.DMAInst)
            and inst.engine == mybir.EngineType.Pool
        ):
            q = getattr(inst, "queue_num", 0) or 0
            if not hasattr(self, "_swq_rot"):
                self._swq_rot = {}
            rot = self._swq_rot.get(q, 0)
            self._swq_rot[q] = rot ^ 1
            lane = (2 * q + rot) % self.swdge_sem_count
            save = self.next_sw_dma_idx
            self.next_sw_dma_idx = lane
            try:
                return orig(self, inst)
            finally:
                self.next_sw_dma_idx = save
        return orig(self, inst)

    cls._assign_tick = _assign_tick
    cls._swq_patched = True


def _install_prof_hook():
    """Shim antenv.axon_hooks so trace=True can NTFF-profile under axon."""
    import contextlib
    import ctypes
    import types

    import antenv

    if "antenv.axon_hooks" in sys.modules:
        return
    so = "/opt/axon/libaxon_pjrt.so"
    lib = ctypes.CDLL(so)
    if not hasattr(lib, "axon_start_nrt_profile"):
        return
    lib.axon_start_nrt_profile.argtypes = [ctypes.POINTER(ctypes.c_int64),
                                           ctypes.c_size_t]
    lib.axon_start_nrt_profile.restype = ctypes.c_int64
    lib.axon_stop_nrt_profile.argtypes = [ctypes.c_char_p]
    lib.axon_stop_nrt_profile.restype = ctypes.c_int64

    @contextlib.contextmanager
    def _hook(output_dir, device_ids):
        import jax

        jax.devices()
        if device_ids:
            ids = (ctypes.c_int64 * len(device_ids))(*device_ids)
            rc = lib.axon_start_nrt_profile(ids, len(device_ids))
        else:
            rc = lib.axon_start_nrt_profile(None, 0)
        if rc != 0:
            raise RuntimeError(f"axon_start_nrt_profile rc={rc}")
        try:
            yield
        finally:
            n = lib.axon_stop_nrt_profile(str(output_dir).encode())
            print(f"profile: {n} file(s) written to {output_dir}",
                  file=sys.stderr)

    mod = types.ModuleType("antenv.axon_hooks")
    mod.get_axon_ntff_profile_hook = lambda: _hook
    mod.set_axon_ntff_profile_hook = lambda h: None
    sys.modules["antenv.axon_hooks"] = mod
    antenv.axon_hooks = mod


# revision 22
# speedup vs baseline: 1.1147x; 1.0268x over previous
"""Trainium2 Bass kernel for nn_AttnAggregator (GNN message passing, 8 cores).

Strategy: data-parallel over queries. Each of the 8 NeuronCores owns 256
queries = 2560 segments. Per core, neighbors are grouped into 20 windows of
128 segments; each window's neighbor list is padded to a fixed number of
128-slot tiles (T, uniform across cores so the SPMD program is identical).

The entity/relation tables are shipped twice: f32 (setup path) and f16. The
main loop gathers neighbor embeddings directly in fp16 (halving gather HBM
traffic), DMA-transposes them for the z GEMM, and keeps every SBUF tensor in
fp16 so DVE ops hit the 2x/4x perf modes and PE matmuls avoid f32r.

The z GEMM runs TRANSPOSED (zT[h, slot] with W1 chunks stationary) so the
score reduction over h becomes per-tile PE dot products against v (1-col
moving operands) instead of DVE tensor work, and exp is one tiny ACT op.

Pipeline per window w (emission is software-pipelined; at iteration `it`):
  S0(it):    dma_gather em16 fp16 (3 parts, SWDGE q0-3) + 2 xbar transposes
             per part -> emT16 [h-chunk, slot] (+ ss/rr gathers every 5 wins)
  A2(it-3):  exp(sc_psum) -> ebuf; wm[t] = (iota==segl)*e  (DVE)
  A1(it-2):  zT = W1.T @ emT + c[q].T (PE fp16, 512-slot PSUM groups) ->
             tanh (ACT, fp16) -> score[t] = tanhT[:,t].T @ v (PE, PSUM)
  PEB(it-3): agg += wm.T @ em16 ; den += wm.T @ ones  (PE fp16 into PSUM)
  C(it-3):   den+inv, recip (DVE); out assembly agg*dinv | ss*mask | rr*mask
             (ACT); DMA out

The c-table c[q] = s_emb[q] @ W2 + r_emb[q] @ W3 + b is computed on-device in
a small fp32 setup phase (gather + PE transpose + matmuls).
"""

import os
import sys

import numpy as np

H = 256
SEQ_LEN = 10
NCORES = 8
WIN = 128  # segments per output window (PSUM partition dim)
SWB = 5    # windows per ss/rr gather batch


def _wrap_idx(idx_lin):
    """Wrap a linear int16 index list for dma_gather: idx i lives at
    [i % 16, i // 16], replicated across the 8 GPSIMD cores (128 rows)."""
    n = len(idx_lin)
    assert n % 16 == 0
    arr = np.asarray(idx_lin, dtype=np.int16).reshape(n // 16, 16).T  # [16, n//16]
    return np.tile(arr, (8, 1)).copy()  # [128, n//16]


def _build_core_data(c, s, r, nbr_ids, seg_ids, QPC, NW):
    """Pure-integer host-side layout work for one core's shard."""
    qlo = c * QPC
    seg_lo = qlo * SEQ_LEN
    seg_hi = (qlo + QPC) * SEQ_LEN
    lo = np.searchsorted(seg_ids, seg_lo, "left")
    hi = np.searchsorted(seg_ids, seg_hi, "left")
    segs = (seg_ids[lo:hi] - seg_lo).astype(np.int64)  # 0 .. QPC*SEQ_LEN-1
    nbrs = nbr_ids[lo:hi].astype(np.int64)

    win_bounds = [np.searchsorted(segs, w * WIN, "left") for w in range(NW + 1)]
    cnts = [win_bounds[w + 1] - win_bounds[w] for w in range(NW)]
    tiles = [max(1, -(-cnt // 128)) for cnt in cnts]
    return segs, nbrs, win_bounds, cnts, tiles


def kernel(s, r, nbr_ids, seg_ids, ent_embeds, rel_embeds, W_attn, b_attn, v_s):
    sys.path.insert(0, "/opt/trn_rl_repo")
    import concourse.bass as bass  # noqa: F401
    import concourse.tile as tile
    from concourse import bacc, mybir
    from concourse.bass_utils import run_bass_kernel_spmd
    from contextlib import ExitStack

    f32 = mybir.dt.float32
    f32r = mybir.dt.float32r
    f16 = mybir.dt.float16
    i16 = mybir.dt.int16
    AF = mybir.ActivationFunctionType
    OP = mybir.AluOpType

    s = np.asarray(s)
    r = np.asarray(r)
    nbr_ids = np.asarray(nbr_ids)
    seg_ids = np.asarray(seg_ids)
    ent_embeds = np.ascontiguousarray(np.asarray(ent_embeds, dtype=np.float32))
    rel_embeds = np.ascontiguousarray(np.asarray(rel_embeds, dtype=np.float32))
    W_attn = np.asarray(W_attn, dtype=np.float32)
    b_attn = np.asarray(b_attn, dtype=np.float32)
    v_s = np.asarray(v_s, dtype=np.float32).reshape(-1)

    ent16_tab = ent_embeds.astype(np.float16)
    rel16_tab = rel_embeds.astype(np.float16)

    B = s.shape[0]
    NUM_SEG = B * SEQ_LEN
    QPC = B // NCORES              # queries per core
    SPC = QPC * SEQ_LEN            # segments per core
    NW = SPC // WIN                # windows per core

    # ---------------- host-side integer layout ----------------
    per_core = [
        _build_core_data(c, s, r, nbr_ids, seg_ids, QPC, NW) for c in range(NCORES)
    ]
    T = max(max(t) for (_, _, _, _, t) in per_core)  # tiles per window (uniform)
    SLOTS = NW * T * 128

    counts_all = np.bincount(np.asarray(seg_ids, dtype=np.int64), minlength=NUM_SEG)

    # Per-WINDOW query base for the c-add one-hot matmul (uniform across
    # cores: computed from w alone). Window w covers local queries
    # [floor(w*128/10), floor(((w+1)*128-1)/10)] — span <= 14 = KQW.
    QW = []  # (qbase, kq) per window
    KQW = WIN // SEQ_LEN + 2  # 14: max queries touched by one window
    for w in range(NW):
        qb = (w * WIN) // SEQ_LEN
        kq = min(KQW, QPC - qb)
        QW.append((qb, kq))

    in_maps = []
    for c in range(NCORES):
        segs, nbrs, wb, cnts, _tiles = per_core[c]
        em_idx = np.zeros(SLOTS, dtype=np.int64)
        segloc = np.full((NW * T, 128), 255.0, dtype=np.float32)  # [tile, part]
        qloc = np.full(SLOTS, -1, dtype=np.int64)
        for w in range(NW):
            cnt = cnts[w]
            base = w * T * 128
            em_idx[base : base + cnt] = nbrs[wb[w] : wb[w + 1]]
            sl = segs[wb[w] : wb[w + 1]] - w * WIN
            tl = np.full(T * 128, 255.0, dtype=np.float32)
            tl[:cnt] = sl.astype(np.float32)
            segloc[w * T : (w + 1) * T, :] = tl.reshape(T, 128)
            qloc[base : base + cnt] = (segs[wb[w] : wb[w + 1]] // SEQ_LEN)

        qoh = np.zeros((KQW, SLOTS), dtype=np.float16)
        for w in range(NW):
            qb = QW[w][0]
            sl = slice(w * T * 128, (w + 1) * T * 128)
            ql = qloc[sl]
            rel_q = np.where(ql >= 0, ql - qb, -1)
            assert rel_q.max() < KQW
            for k in range(KQW):
                qoh[k, sl] = (rel_q == k).astype(np.float16)

        # per-segment arrays
        seg_global0 = c * SPC
        segq = (np.arange(SPC) // SEQ_LEN) + c * QPC  # global query per local seg
        sw_idx = s[segq].astype(np.int64)  # ent row per local seg
        rw_idx = r[segq].astype(np.int64)
        cnts_core = counts_all[seg_global0 : seg_global0 + SPC]
        maskw = (cnts_core > 0).astype(np.float32).reshape(NW, 128).T  # [128, NW]
        invw = 1.0 - maskw

        sq = s[c * QPC : (c + 1) * QPC].astype(np.int64)  # [QPC]
        rq = r[c * QPC : (c + 1) * QPC].astype(np.int64)

        im = {
            "ent": ent_embeds,
            "rel": rel_embeds,
            "ent16": ent16_tab,
            "rel16": rel16_tab,
            "wq1z": W_attn[0:256].reshape(2, 128, 2, 128).transpose(1, 0, 2, 3)
                   .astype(np.float16).copy(),
            "wq2": W_attn[256:512].reshape(2, 128, 256).transpose(1, 0, 2).copy(),
            "wq3": W_attn[512:768].reshape(2, 128, 256).transpose(1, 0, 2).copy(),
            "b_row": b_attn.reshape(1, 256).copy(),
            "vcol": np.ascontiguousarray(
                v_s.astype(np.float16).reshape(2, 128).T),
            "ones2": np.ones((128, 2), dtype=np.float16),
            "ones_row": np.ones((1, 128), dtype=np.float32),
            "ident": np.eye(128, dtype=np.float32),
            "iota32": np.tile(np.arange(128, dtype=np.float32), (128, 1)),
            "em_idx": _wrap_idx(em_idx),
            "sq_idx": _wrap_idx(sq),
            "rq_idx": _wrap_idx(rq),
            "sw_idx": _wrap_idx(sw_idx),
            "rw_idx": _wrap_idx(rw_idx),
            "segl": np.ascontiguousarray(segloc.T),
            "qoh": qoh,
            "maskw": np.ascontiguousarray(maskw),
            "invw": np.ascontiguousarray(invw),
        }
        in_maps.append(im)

    # ---------------- build the SPMD program ----------------
    print("[kernel] host prep done", flush=True)
    nc = bacc.Bacc("TRN2", target_bir_lowering=False, debug=False,
                   num_devices=NCORES, num_swdge_queues=4)

    def din(name, shape, dt):
        return nc.dram_tensor(name, shape, dt, kind="ExternalInput").ap()

    ent_ap = din("ent", [ent_embeds.shape[0], 256], f32)
    rel_ap = din("rel", [rel_embeds.shape[0], 256], f32)
    ent16_ap = din("ent16", [ent_embeds.shape[0], 256], f16)
    rel16_ap = din("rel16", [rel_embeds.shape[0], 256], f16)
    wq1z_ap = din("wq1z", [128, 2, 2, 128], f16)
    wq2_ap = din("wq2", [128, 2, 256], f32)
    wq3_ap = din("wq3", [128, 2, 256], f32)
    brow_ap = din("b_row", [1, 256], f32)
    vcol_ap = din("vcol", [128, 2], f16)
    ones2_ap = din("ones2", [128, 2], f16)
    onesr_ap = din("ones_row", [1, 128], f32)
    ident_ap = din("ident", [128, 128], f32)
    iota32_ap = din("iota32", [128, 128], f32)
    emidx_ap = din("em_idx", [128, SLOTS // 16], i16)
    sqidx_ap = din("sq_idx", [128, QPC // 16], i16)
    rqidx_ap = din("rq_idx", [128, QPC // 16], i16)
    swidx_ap = din("sw_idx", [128, SPC // 16], i16)
    rwidx_ap = din("rw_idx", [128, SPC // 16], i16)
    segl_ap = din("segl", [128, NW * T], f32)
    qoh_ap = din("qoh", [KQW, SLOTS], f16)
    maskw_ap = din("maskw", [128, NW], f32)
    invw_ap = din("invw", [128, NW], f32)
    out_ap = nc.dram_tensor("out", [SPC, 768], f32, kind="ExternalOutput").ap()

    import itertools as _it
    _swq_counter = _it.count()

    def _swq():
        return next(_swq_counter) % 4

    _patch_swdge_lane_assignment()

    import time as _time
    _t0 = _time.time()
    with tile.TileContext(nc) as tc, ExitStack() as ctx:
        cons = ctx.enter_context(tc.tile_pool(name="cons", bufs=1))
        emp = ctx.enter_context(tc.tile_pool(name="emp", bufs=4))
        emq = ctx.enter_context(tc.tile_pool(name="emq", bufs=3))
        hvp = ctx.enter_context(tc.tile_pool(name="hvp", bufs=2))
        wmp = ctx.enter_context(tc.tile_pool(name="wmp", bufs=2))
        work = ctx.enter_context(tc.tile_pool(name="work", bufs=2))
        outp = ctx.enter_context(tc.tile_pool(name="outp", bufs=2))
        ps_z = ctx.enter_context(tc.tile_pool(name="ps_z", bufs=2, space="PSUM"))
        ps_a = ctx.enter_context(tc.tile_pool(name="ps_a", bufs=2, space="PSUM"))
        ps_d = ctx.enter_context(tc.tile_pool(name="ps_d", bufs=1, space="PSUM"))
        ps_s = ctx.enter_context(tc.tile_pool(name="ps_s", bufs=1, space="PSUM"))

        # resident constants
        def cload(tag, shape, dt, ap, cast=False):
            t = cons.tile(shape, dt, tag=tag)
            nc.sync.dma_start(t[:], (ap.bitcast(dt) if cast else ap)[:])
            return t

        wq1z = cload("wq1z", [128, 2, 2, 128], f16, wq1z_ap)
        wq2 = cload("wq2", [128, 2, 256], f32r, wq2_ap, cast=True)
        wq3 = cload("wq3", [128, 2, 256], f32r, wq3_ap, cast=True)
        brow = cload("brow", [1, 256], f32r, brow_ap, cast=True)
        vcol = cload("vcol", [128, 2], f16, vcol_ap)
        onesr = cload("onesr", [1, 128], f32r, onesr_ap, cast=True)
        ident = cload("ident", [128, 128], f32r, ident_ap, cast=True)
        iota32 = cload("iota32", [128, 128], f32, iota32_ap)
        segl = cload("segl", [128, NW * T], f32, segl_ap)
        ones2 = cload("ones2", [128, 2], f16, ones2_ap)
        emidx = cload("emidx", [128, SLOTS // 16], i16, emidx_ap)
        sqidx = cload("sqidx", [128, QPC // 16], i16, sqidx_ap)
        rqidx = cload("rqidx", [128, QPC // 16], i16, rqidx_ap)
        swidx = cload("swidx", [128, SPC // 16], i16, swidx_ap)
        rwidx = cload("rwidx", [128, SPC // 16], i16, rwidx_ap)
        maskw = cload("maskw", [128, NW], f32, maskw_ap)
        invw = cload("invw", [128, NW], f32, invw_ap)

        # ---- setup: c-table c[q] = s_emb[q] @ W2 + r_emb[q] @ W3 + b ----
        # (gathers issued up front; compute emitted after the first two
        # windows' S0 so it overlaps their gather transfers)
        s_emb = cons.tile([128, QPC // 128, 256], f32r)
        nc.gpsimd.dma_gather(s_emb[:], ent_ap.bitcast(f32r)[:], sqidx[:],
                             num_idxs=QPC, num_idxs_reg=QPC, elem_size=256,
                             single_packet=False, queue_num=_swq())
        r_emb = cons.tile([128, QPC // 128, 256], f32r)
        nc.gpsimd.dma_gather(r_emb[:], rel_ap.bitcast(f32r)[:], rqidx[:],
                             num_idxs=QPC, num_idxs_reg=QPC, elem_size=256,
                             single_packet=False, queue_num=_swq())

        def setup_ctable():
            sT = cons.tile([128, 2, 256], f32r)   # [h, hc, q]
            rT = cons.tile([128, 2, 256], f32r)
            for gsrc, dstT in ((s_emb, sT), (r_emb, rT)):
                tp = ps_a.tile([128, 2, 256], f32r, tag="agg")
                for qc in range(2):
                    for hc in range(2):
                        nc.tensor.transpose(tp[:, hc, qc * 128:(qc + 1) * 128],
                                            gsrc[:, qc, hc * 128:(hc + 1) * 128],
                                            ident[:])
                nc.scalar.copy(dstT[:], tp[:])

            cw = cons.tile([32, NW, 256], f16)
            for w in range(NW):
                qb, kq = QW[w]
                cp = ps_z.tile([128, 4, 256], f32, tag="z")
                for hc in range(2):
                    nc.tensor.matmul(cp[0:kq, 0, :], sT[:, hc, qb:qb + kq],
                                     wq2[:, hc, :], start=(hc == 0), stop=False)
                for hc in range(2):
                    nc.tensor.matmul(cp[0:kq, 0, :], rT[:, hc, qb:qb + kq],
                                     wq3[:, hc, :], start=False, stop=False)
                nc.tensor.matmul(cp[0:kq, 0, :], onesr[:, 0:kq], brow[:],
                                 start=False, stop=True)
                nc.scalar.copy(cw[0:kq, w, :], cp[0:kq, 0, :])
            return cw

        # ---- software-pipelined main loop over windows ----
        NW_RUN = int(os.environ.get("KERNEL_NWIN", str(NW)))
        NQ = 3
        tparts = [(T * p // NQ, T * (p + 1) // NQ) for p in range(NQ)]
        ngrp = (T + 3) // 4

        em16_t = {}
        emT_t = {}
        qoh_t = {}
        sc_t = {}
        th_t = {}
        wm_t = {}
        agg_t = {}
        den_t = {}
        ssb_t = {}
        rrb_t = {}
        osb_t = {}

        def S0(w):
            """Gather + transpose + one-hot load for window w."""
            em16 = emp.tile([128, T, 256], f16, tag="em16", bufs=5)
            emT = emq.tile([128, T, 2, 128], f16, tag="emT", bufs=4)
            for tlo, thi in tparts:
                nt = thi - tlo
                nc.gpsimd.dma_gather(
                    em16[:, tlo:thi, :], ent16_ap[:],
                    emidx[:, (w * T + tlo) * 8:(w * T + thi) * 8],
                    num_idxs=nt * 128, num_idxs_reg=nt * 128, elem_size=256,
                    single_packet=False, queue_num=_swq())
                nc.sync.dma_start(emT[:, tlo:thi, :, :],
                                  em16[:, tlo:thi, :], transpose=True)
            qoh_w = emq.tile([KQW, T * 128], f16, tag="qoh", bufs=4)
            nc.sync.dma_start(qoh_w[:],
                              qoh_ap[:, w * T * 128:(w + 1) * T * 128])
            em16_t[w] = em16
            emT_t[w] = emT
            qoh_t[w] = qoh_w
            if w % SWB == 0:
                nsw = min(SWB, NW_RUN - w)
                ssb = outp.tile([128, SWB, 256], f16, tag="ssb", bufs=2)
                nc.gpsimd.dma_gather(
                    ssb[:, 0:nsw, :], ent16_ap[:],
                    swidx[:, w * 8:(w + nsw) * 8],
                    num_idxs=nsw * 128, num_idxs_reg=nsw * 128, elem_size=256,
                    single_packet=False, queue_num=_swq())
                rrb = outp.tile([128, SWB, 256], f16, tag="rrb", bufs=2)
                nc.gpsimd.dma_gather(
                    rrb[:, 0:nsw, :], rel16_ap[:],
                    rwidx[:, w * 8:(w + nsw) * 8],
                    num_idxs=nsw * 128, num_idxs_reg=nsw * 128, elem_size=256,
                    single_packet=False, queue_num=_swq())
                ssb_t[w // SWB] = ssb
                rrb_t[w // SWB] = rrb

        def A1(w):
            """zT GEMM + tanh + PE score dot-products for window w."""
            qb, kq = QW[w]
            emT = emT_t.pop(w)
            qoh_w = qoh_t.pop(w)
            tanhT = hvp.tile([128, 2, T * 128], f16, tag="tanhT", bufs=2)
            sc_ps = ps_s.tile([128, T], f32, tag="sc")

            def score_mm(g):
                t0 = g * 4
                for t in range(t0, min(t0 + 4, T)):
                    for hc in range(2):
                        nc.tensor.matmul(sc_ps[:, t:t + 1],
                                         tanhT[:, hc, t * 128:(t + 1) * 128],
                                         vcol[:, hc:hc + 1],
                                         start=(hc == 0), stop=(hc == 1))

            for g in range(ngrp):
                t0 = g * 4
                nt = min(4, T - t0)
                sl = slice(t0 * 128, (t0 + nt) * 128)
                zp = ps_z.tile([128, 2, 512], f32, tag="z")
                for hc in range(2):
                    zps = zp[:, hc, 0:nt * 128]
                    for kc in range(2):
                        nc.tensor.matmul(zps, wq1z[:, kc, hc, :],
                                         emT[:, t0:t0 + nt, kc, :],
                                         start=(kc == 0), stop=False)
                    nc.tensor.matmul(zps,
                                     c_win[0:kq, w, hc * 128:(hc + 1) * 128],
                                     qoh_w[0:kq, sl],
                                     start=False, stop=True)
                nc.scalar.activation(tanhT[:, :, sl], zp[:, :, 0:nt * 128],
                                     AF.Tanh)
                if g > 0:
                    score_mm(g - 1)
            score_mm(ngrp - 1)
            sc_t[w] = sc_ps

        def A1s(w):
            pass

        def A2(w):
            """exp + per-tile weight-mask generation for window w."""
            sc_ps = sc_t.pop(w)
            ebuf = wmp.tile([128, T], f32, tag="ebuf", bufs=2)
            nc.scalar.activation(ebuf[:], sc_ps[:], AF.Exp)
            wm = wmp.tile([128, T, 128], f16, tag="wm", bufs=2)
            for t in range(T):
                e_b, _ = bass.broadcast_tensor_aps(ebuf[:, t:t + 1], iota32[:])
                nc.vector.scalar_tensor_tensor(
                    wm[:, t, :], iota32[:],
                    segl[:, w * T + t:w * T + t + 1], e_b,
                    op0=OP.is_equal, op1=OP.mult)
            wm_t[w] = wm

        def PEB(w):
            """Scatter matmuls for window w."""
            wm = wm_t.pop(w)
            em16 = em16_t.pop(w)
            agg_ps = ps_a.tile([128, 256], f32, tag="agg")
            den_ps = ps_d.tile([128, 2], f32, tag="den")
            for t in range(T):
                nc.tensor.matmul(agg_ps[:], wm[:, t, :], em16[:, t, :],
                                 start=(t == 0), stop=(t == T - 1))
                nc.tensor.matmul(den_ps[:], wm[:, t, :], ones2[:],
                                 start=(t == 0), stop=(t == T - 1))
            agg_t[w] = agg_ps
            den_t[w] = den_ps

        def C(w):
            """Normalize + assemble + write out window w."""
            agg_ps = agg_t.pop(w)
            den_ps = den_t.pop(w)
            dtmp = work.tile([128, 1], f32, tag="dtmp", bufs=2)
            nc.vector.tensor_add(dtmp[:], den_ps[:, 0:1], invw[:, w:w + 1])
            dinv = work.tile([128, 1], f32, tag="dinv", bufs=2)
            nc.vector.reciprocal(dinv[:], dtmp[:])
            out_sb = outp.tile([128, 768], f32, tag="out", bufs=4)
            nc.scalar.activation(out_sb[:, 0:256], agg_ps[:, 0:256], AF.Copy,
                                 scale=dinv[:])
            b, k = w // SWB, w % SWB
            nc.scalar.activation(out_sb[:, 256:512], ssb_t[b][:, k, :],
                                 AF.Copy, scale=maskw[:, w:w + 1])
            nc.scalar.activation(out_sb[:, 512:768], rrb_t[b][:, k, :],
                                 AF.Copy, scale=maskw[:, w:w + 1])
            osb_t[w] = out_sb

        def CDMA(w):
            """Write out window w (emitted before next transposes on Sync)."""
            out_sb = osb_t.pop(w)
            nc.sync.dma_start(out_ap[w * 128:(w + 1) * 128, :], out_sb[:])

        c_win = None
        for it in range(NW_RUN + 5):
            if 0 <= it - 4 < NW_RUN:
                CDMA(it - 4)
            if it < NW_RUN:
                S0(it)
            if it == min(1, NW_RUN - 1):
                c_win = setup_ctable()
            if 0 <= it - 3 < NW_RUN:
                A2(it - 3)
            if 0 <= it - 2 < NW_RUN:
                A1(it - 2)
            if 0 <= it - 3 < NW_RUN:
                PEB(it - 3)
            if 0 <= it - 2 < NW_RUN:
                A1s(it - 2)
            if 0 <= it - 3 < NW_RUN:
                C(it - 3)

    print(f"[kernel] program built+scheduled in {_time.time()-_t0:.1f}s",
          flush=True)
    nc.compile()
    print("[kernel] bacc.compile done; launching", flush=True)

    if os.environ.get("KERNEL_SIM"):
        from concourse.bass_interp import CoreSim
        sim = CoreSim(nc, trace=False)
        for k, v in in_maps[0].items():
            sim.tensor(k)[:] = v
        sim.simulate(check_with_hw=False)
        print("[kernel] CoreSim passed", flush=True)
        import types
        res = types.SimpleNamespace(
            results=[{"out": np.array(sim.tensor("out"))} for _ in range(NCORES)],
            exec_time_ns=None)
        out = np.concatenate([res.results[c]["out"] for c in range(NCORES)], axis=0)
        return out.reshape(B, SEQ_LEN, 3 * H)

    trace = bool(int(os.environ.get("KERNEL_TRACE", "0")))
    if trace:
        _install_prof_hook()
    res = run_bass_kernel_spmd(nc, in_maps, list(range(NCORES)), trace=trace)
    if trace and res.exec_time_ns is not None:
        print(f"HW exec time: {res.exec_time_ns} ns")

    out = np.concatenate([res.results[c]["out"] for c in range(NCORES)], axis=0)
    return out.reshape(B, SEQ_LEN, 3 * H)


def _patch_swdge_lane_assignment():
    """Make Tile's DMASW completion-sem lane choice queue-aware so SWDGE
    multi-queue DMAs don't share a semaphore lane across queues (each sem is
    locked to the queue that first increments it). Lanes 2q and 2q+1 serve
    queue q."""
    import concourse.tile_sem_assignment as tsa
    import concourse.mybir as mybir

    cls = tsa.TileClockTick
    if getattr(cls, "_swq_patched", False):
        return
    orig = cls._assign_tick

    def _assign_tick(self, inst):
        if (
            isinstance(inst, tsa.DMAInst)
            and inst.engine == mybir.EngineType.Pool
        ):
            q = getattr(inst, "queue_num", 0) or 0
            if not hasattr(self, "_swq_rot"):
                self._swq_rot = {}
            rot = self._swq_rot.get(q, 0)
            self._swq_rot[q] = rot ^ 1
            lane = (2 * q + rot) % self.swdge_sem_count
            save = self.next_sw_dma_idx
            self.next_sw_dma_idx = lane
            try:
                return orig(self, inst)
            finally:
                self.next_sw_dma_idx = save
        return orig(self, inst)

    cls._assign_tick = _assign_tick
    cls._swq_patched = True


def _install_prof_hook():
    """Shim antenv.axon_hooks so trace=True can NTFF-profile under axon."""
    import contextlib
    import ctypes
    import types

    import antenv

    if "antenv.axon_hooks" in sys.modules:
        return
    so = "/opt/axon/libaxon_pjrt.so"
    lib = ctypes.CDLL(so)
    if not hasattr(lib, "axon_start_nrt_profile"):
        return
    lib.axon_start_nrt_profile.argtypes = [ctypes.POINTER(ctypes.c_int64),
                                           ctypes.c_size_t]
    lib.axon_start_nrt_profile.restype = ctypes.c_int64
    lib.axon_stop_nrt_profile.argtypes = [ctypes.c_char_p]
    lib.axon_stop_nrt_profile.restype = ctypes.c_int64

    @contextlib.contextmanager
    def _hook(output_dir, device_ids):
        import jax

        jax.devices()
        if device_ids:
            ids = (ctypes.c_int64 * len(device_ids))(*device_ids)
            rc = lib.axon_start_nrt_profile(ids, len(device_ids))
        else:
            rc = lib.axon_start_nrt_profile(None, 0)
        if rc != 0:
            raise RuntimeError(f"axon_start_nrt_profile rc={rc}")
        try:
            yield
        finally:
            n = lib.axon_stop_nrt_profile(str(output_dir).encode())
            print(f"profile: {n} file(s) written to {output_dir}",
                  file=sys.stderr)

    mod = types.ModuleType("antenv.axon_hooks")
    mod.get_axon_ntff_profile_hook = lambda: _hook
    mod.set_axon_ntff_profile_hook = lambda h: None
    sys.modules["antenv.axon_hooks"] = mod
    antenv.axon_hooks = mod
